# revision 23
# baseline (speedup 1.0000x reference)
"""DiT block kernel for 8 Trainium2 NeuronCores.

Strategy: pure data-parallel over batch (16 batches -> 2 per core), activations
feature-major ([feature, token]). Big GEMMs (qkv, v, proj, mlp1, mlp2) run as
fp8e4 DoubleRow matmuls (0.5 cycles/row, 256-deep contraction) with weights
host-scaled by 16 to dodge fp8 subnormals; the 1/16 is folded into the PSUM
evacuation (Act scale or scalar_tensor_tensor scalar). Contraction dims are
zero-padded to a multiple of 256 (H: 9 chunks -> 10). The adaLN modulation is
computed feature-major in bf16 (tiny output, gates need precision), giving
per-partition scale/bias columns for free. LayerNorm / softmax reductions run
as ones/indicator matmuls on the PE (fp32r); per-token stats broadcast back via
K=1 expansion matmuls. Softmax normalization is deferred through attn@v via an
appended ones-column on v. MLP1->MLP2 are fused per 512-token block (hidden
stays in SBUF as fp8).
"""

import sys

sys.path.insert(0, "/opt/trn_rl_repo")

from contextlib import ExitStack

import ml_dtypes
import numpy as np

import concourse.bacc as bacc
import concourse.tile as tile
from concourse import mybir
from concourse.bass_utils import run_bass_kernel_spmd

FP32 = mybir.dt.float32
FP32R = mybir.dt.float32r
BF16 = mybir.dt.bfloat16
FP8 = mybir.dt.float8e4
AF = mybir.ActivationFunctionType
ALU = mybir.AluOpType
DR = mybir.MatmulPerfMode.DoubleRow

B, N, H = 16, 1024, 1152
NH, HD = 16, 72
MLP = H * 4
NCORES = 8
BPC = B // NCORES            # batches per core = 2
T = BPC * N                  # tokens per core = 2048
NTC = T // 512               # token chunks of 512 = 4
FK = H // 128                # 9
NCH = FK + 1                 # padded contraction chunks for H = 10
NJH = NCH // 2               # DoubleRow pair-steps for H = 5
MK = MLP // 128              # 36
NJM = MK // 2                # DoubleRow pair-steps for MLP = 18
QKF = (2 * H) // 128         # 18 qk feature tiles
EPS = 1e-6
ISQ = float(HD) ** -0.5
SCL = 16.0                   # fp8 weight scale
ISCL = 1.0 / SCL

_CACHE = {}
_DBG_REFS = {}
PHASES = 8  # build phases up to this number (bisect aid)
DBG = None  # None | "mod" | "xn" | "qk" | "v" | "o" | "x2" | "xn2": dump intermediate to out


def _build_program():
    nc = bacc.Bacc("TRN2", target_bir_lowering=False, debug=False)

    xT = nc.dram_tensor("xT", [H, T], FP32R, kind="ExternalInput")
    cT = nc.dram_tensor("cT", [H, BPC], FP32, kind="ExternalInput")
    wmodb = nc.dram_tensor("wmodb", [H, 6 * H], BF16, kind="ExternalInput")
    bmodc = nc.dram_tensor("bmodc", [128, 54], FP32, kind="ExternalInput")
    wqk8 = nc.dram_tensor("wqk8", [128, NJH * 2 * 2 * H], FP8, kind="ExternalInput")
    wv8 = nc.dram_tensor("wv8", [128, NJH * 2 * H], FP8, kind="ExternalInput")
    bqk = nc.dram_tensor("bqk", [128, QKF], FP32, kind="ExternalInput")
    bv8 = nc.dram_tensor("bv8", [1, H], FP8, kind="ExternalInput")
    wpro8 = nc.dram_tensor("wpro8", [128, NJH * 2 * H], FP8, kind="ExternalInput")
    bpro8 = nc.dram_tensor("bpro8", [1, H], FP8, kind="ExternalInput")
    w18 = nc.dram_tensor("w18", [128, NJH * 2 * MLP], FP8, kind="ExternalInput")
    b1c = nc.dram_tensor("b1c", [128, MK], FP32, kind="ExternalInput")
    w28 = nc.dram_tensor("w28", [128, NJM * 2 * H], FP8, kind="ExternalInput")
    b28 = nc.dram_tensor("b28", [1, H], FP8, kind="ExternalInput")
    inds = nc.dram_tensor("inds", [128, QKF * 64], FP32R, kind="ExternalInput")
    indq = nc.dram_tensor("indq", [128, QKF * 64], FP32R, kind="ExternalInput")
    indh = nc.dram_tensor("indh", [32, 32 * HD], FP32R, kind="ExternalInput")
    onesr = nc.dram_tensor("onesr", [1, 512], FP32R, kind="ExternalInput")
    ones8r = nc.dram_tensor("ones8r", [1, 512], FP8, kind="ExternalInput")
    onesc = nc.dram_tensor("onesc", [128, 1], FP32R, kind="ExternalInput")
    epsc = nc.dram_tensor("epsc", [128, 1], FP32, kind="ExternalInput")
    vtail = nc.dram_tensor("vtail", [128, 25], FP32R, kind="ExternalInput")
    out = nc.dram_tensor("out", [H, T], FP32, kind="ExternalOutput")

    with nc.allow_low_precision(
        reason="fp8 DoubleRow matmuls with x16 weight scaling; errors diluted "
        "by the residual stream, gate 2e-2 has big margin"
    ), tile.TileContext(nc) as tc, ExitStack() as top:
        dram = top.enter_context(tc.tile_pool(name="dram", bufs=1, space="DRAM"))
        qk_d = dram.tile([2 * H, T], FP32R, tag="qk_d", name="qk_d")
        v_d = dram.tile([T, H], FP8, tag="v_d", name="v_d")
        o_d = dram.tile([H, T], FP8, tag="o_d", name="o_d")
        x2_d = dram.tile([H, T], FP32R, tag="x2_d", name="x2_d")

        cst = top.enter_context(tc.tile_pool(name="cst", bufs=1))
        ones512 = cst.tile([1, 512], FP32R, tag="o512", name="o512")
        nc.sync.dma_start(ones512[:], onesr.ap())
        ones512_8 = cst.tile([1, 512], FP8, tag="o512f8", name="o512f8")
        nc.sync.dma_start(ones512_8[:], ones8r.ap())
        ocol128 = cst.tile([128, 1], FP32R, tag="oc128", name="oc128")
        nc.sync.dma_start(ocol128[:], onesc.ap())
        eps_col = cst.tile([128, 1], FP32, tag="epsc", name="epsc")
        nc.sync.dma_start(eps_col[:], epsc.ap())
        vtail_s = cst.tile([128, 25], FP32R, tag="vtl", name="vtl")
        nc.sync.dma_start(vtail_s[:], vtail.ap())
        neg2 = cst.tile([128, 1], FP32, tag="neg2", name="neg2")
        nc.vector.memset(neg2[:], -2.0)
        ones1x1 = ones512[0:1, 0:1]
        ones72 = ones512[0:1, 0:HD]

        bqk_s = cst.tile([128, QKF], FP32, tag="bqk", name="bqk")
        nc.sync.dma_start(bqk_s[:], bqk.ap())
        bv8_s = cst.tile([1, H], FP8, tag="bv8", name="bv8")
        nc.sync.dma_start(bv8_s[:], bv8.ap())
        bpro8_s = cst.tile([1, H], FP8, tag="bpro8", name="bpro8")
        nc.sync.dma_start(bpro8_s[:], bpro8.ap())
        b1_s = cst.tile([128, MK], FP32, tag="b1", name="b1")
        nc.sync.dma_start(b1_s[:], b1c.ap())
        b28_s = cst.tile([1, H], FP8, tag="b28", name="b28")
        nc.sync.dma_start(b28_s[:], b28.ap())
        bmodc_s = cst.tile([128, 54], FP32, tag="bmodc", name="bmodc")
        nc.sync.dma_start(bmodc_s[:], bmodc.ap())

        gcol_p = top.enter_context(tc.tile_pool(name="gcolp", bufs=1))
        gcol16 = [[gcol_p.tile([128, FK], FP32, tag=f"gc{u}{b}", name=f"gc{u}{b}")
                   for b in range(BPC)] for u in range(2)]

        mod_p = top.enter_context(tc.tile_pool(name="modp", bufs=1))
        sc_t = [[mod_p.tile([1, H], FP32R, tag=f"sc{u}{b}", name=f"sc{u}{b}") for b in range(BPC)]
                for u in range(2)]
        sh_t = [[mod_p.tile([1, H], FP32R, tag=f"sh{u}{b}", name=f"sh{u}{b}") for b in range(BPC)]
                for u in range(2)]

        stq_p = tc.alloc_tile_pool(name="stqp", bufs=1)
        stq_r = stq_p.tile([32, T], FP32R, tag="stq_r", name="stq_r")
        stq_mr = stq_p.tile([32, T], FP32R, tag="stq_mr", name="stq_mr")

        # ---------------- phase 0: adaLN modulation (feature-major, bf16) ----
        with ExitStack() as ph:
            sb = ph.enter_context(tc.tile_pool(name="p0sb", bufs=2))
            wm = ph.enter_context(tc.tile_pool(name="p0wm", bufs=2))
            ps = ph.enter_context(tc.tile_pool(name="p0ps", bufs=1, space="PSUM"))
            modc = mod_p.tile([128, 108], FP32, tag="modc", name="modc")
            _DBG_REFS["modc"] = modc
            swcb = mod_p.tile([128, 2 * FK], BF16, tag="swcb", name="swcb")
            for k in range(FK):
                craw = sb.tile([128, BPC], FP32, tag="craw", name="craw")
                nc.sync.dma_start(craw[:], cT.ap()[k * 128:(k + 1) * 128, :])
                nc.scalar.activation(swcb[:, 2 * k:2 * k + 2], craw[:], AF.Silu,
                                     bias=0.0, scale=1.0)
            pm = ps.tile([128, 108], FP32, tag="pm", name="pm")
            for k in range(FK):
                wmk = wm.tile([128, 6 * H], BF16, tag="wmk", name="wmk")
                nc.sync.dma_start(wmk[:], wmodb.ap()[k * 128:(k + 1) * 128, :])
                for c in range(54):
                    # one accumulation group for the whole 2KB psum region:
                    # start only zeroes once (it clears the full bank row)
                    nc.tensor.matmul(pm[:, 2 * c:2 * c + 2],
                                     wmk[:, c * 128:(c + 1) * 128],
                                     swcb[:, 2 * k:2 * k + 2],
                                     start=(k == 0 and c == 0),
                                     stop=(k == FK - 1 and c == 53),
                                     skip_group_check=True)
            for c in range(54):
                nc.scalar.activation(modc[:, 2 * c:2 * c + 2], pm[:, 2 * c:2 * c + 2],
                                     AF.Identity, bias=bmodc_s[:, c:c + 1], scale=1.0)
            # columns -> row vectors (sc, sh) and gate columns /16
            for u in range(2):
                for b in range(BPC):
                    for ft in range(FK):
                        csc = ((1 + 3 * u) * FK + ft) * 2 + b
                        csh = ((0 + 3 * u) * FK + ft) * 2 + b
                        cg = ((2 + 3 * u) * FK + ft) * 2 + b
                        fsl = slice(ft * 128, (ft + 1) * 128)
                        nc.sync.dma_start(sc_t[u][b][0:1, fsl],
                                          modc[:, csc:csc + 1].bitcast(FP32R))
                        nc.sync.dma_start(sh_t[u][b][0:1, fsl],
                                          modc[:, csh:csh + 1].bitcast(FP32R))
                        nc.vector.tensor_scalar_mul(gcol16[u][b][:, ft:ft + 1],
                                                    modc[:, cg:cg + 1], ISCL)

        # ---------------- LayerNorm + modulation -> fp8 chunk-major ----------
        def ln_modulate(src_dram, u, dst_tiles):
            """dst_tiles[n] is a [128, NCH, 512] fp8 tile; writes chunks 0..8."""
            with ExitStack() as ph:
                sb = ph.enter_context(tc.tile_pool(name="lnsb", bufs=1))
                xp = ph.enter_context(tc.tile_pool(name="lnx", bufs=1))
                ps = ph.enter_context(tc.tile_pool(name="lnps", bufs=2, space="PSUM"))
                pe = ph.enter_context(tc.tile_pool(name="lnpe", bufs=2, space="PSUM"))
                for n in range(NTC):
                    b = n // (NTC // BPC)
                    nsl = slice(n * 512, (n + 1) * 512)
                    xc = [xp.tile([128, 512], FP32R, tag=f"xc{k}", name=f"xc{k}") for k in range(FK)]
                    ln_s = ps.tile([1, 512], FP32, tag="lns", name="lns")
                    ln_q = ps.tile([1, 512], FP32, tag="lnq", name="lnq")
                    for k in range(FK):
                        nc.sync.dma_start(xc[k][:], src_dram[k * 128:(k + 1) * 128, nsl])
                        sq = sb.tile([128, 512], FP32R, tag="sq", name="sq", bufs=3)
                        nc.vector.tensor_mul(sq[:], xc[k][:], xc[k][:])
                        nc.tensor.matmul(ln_s[:], ocol128[:], xc[k][:],
                                         start=(k == 0), stop=(k == FK - 1))
                        nc.tensor.matmul(ln_q[:], ocol128[:], sq[:],
                                         start=(k == 0), stop=(k == FK - 1))
                    ms_s = sb.tile([1, 512], FP32, tag="ms_s", name="ms_s")
                    nc.scalar.mul(ms_s[:], ln_s[:], 1.0 / H)
                    ms_q = sb.tile([1, 512], FP32, tag="ms_q", name="ms_q")
                    nc.scalar.mul(ms_q[:], ln_q[:], 1.0 / H)
                    m2 = sb.tile([1, 512], FP32, tag="m2", name="m2")
                    nc.vector.tensor_mul(m2[:], ms_s[:], ms_s[:])
                    var = sb.tile([1, 512], FP32, tag="var", name="var")
                    nc.vector.tensor_sub(var[:], ms_q[:], m2[:])
                    sd = sb.tile([1, 512], FP32, tag="sd", name="sd")
                    nc.scalar.activation(sd[:], var[:], AF.Sqrt, bias=eps_col[0:1, :], scale=1.0)
                    stA = sb.tile([1, 512], FP32R, tag="stA", name="stA")
                    nc.vector.reciprocal(stA[:], sd[:])
                    stC = sb.tile([1, 512], FP32R, tag="stC", name="stC")
                    mA = sb.tile([1, 512], FP32, tag="mA", name="mA")
                    nc.vector.tensor_mul(mA[:], ms_s[:], stA[:])
                    nc.vector.tensor_scalar_mul(stC[:], mA[:], -1.0)
                    for ft in range(FK):
                        fsl = slice(ft * 128, (ft + 1) * 128)
                        al = pe.tile([128, 512], FP32, tag="al", name="al")
                        nc.tensor.matmul(al[:], sc_t[u][b][0:1, fsl], stA[:],
                                         start=True, stop=True)
                        be = pe.tile([128, 512], FP32, tag="be", name="be")
                        nc.tensor.matmul(be[:], sc_t[u][b][0:1, fsl], stC[:],
                                         start=True, stop=False)
                        nc.tensor.matmul(be[:], sh_t[u][b][0:1, fsl], ones512[:],
                                         start=False, stop=True)
                        tmp = sb.tile([128, 512], FP32, tag="tmp", name="tmp", bufs=3)
                        nc.vector.tensor_tensor(tmp[:], xc[ft][:], al[:], op=ALU.mult)
                        nc.vector.tensor_tensor(dst_tiles[n][:, ft:ft + 1, :], tmp[:],
                                                be[:], op=ALU.add)

        # ---------------- phase 1: LN1 -> xn8; qkv ----------------
        xn_p = tc.alloc_tile_pool(name="xnp", bufs=1)
        xn8 = [xn_p.tile([128, NCH, 512], FP8, tag=f"xn{n}", name=f"xn{n}")
               for n in range(NTC)]
        _DBG_REFS["xn"] = xn8
        for n in range(NTC):
            nc.vector.memset(xn8[n][:, FK:NCH, :], 0.0)
        ln_modulate(xT.ap(), 0, xn8)

        if PHASES >= 3:
            with ExitStack() as ph:
                sb = ph.enter_context(tc.tile_pool(name="qksb", bufs=3))
                wp = ph.enter_context(tc.tile_pool(name="qkw", bufs=1))
                ip = ph.enter_context(tc.tile_pool(name="qki", bufs=2))
                ps = ph.enter_context(tc.tile_pool(name="qkps", bufs=2, space="PSUM"))
                st = ph.enter_context(tc.tile_pool(name="qkst", bufs=1, space="PSUM"))
                qstat = [st.tile([64, 512], FP32, tag=f"qs{n}", name=f"qs{n}") for n in range(NTC)]
                wqkt = [wp.tile([128, 2, 2 * H], FP8, tag=f"wqk{j}", name=f"wqk{j}")
                        for j in range(NJH)]
                for j in range(NJH):
                    nc.sync.dma_start(wqkt[j][:], wqk8.ap()[:, j * 4 * H:(j + 1) * 4 * H])
                for mb in range(QKF):
                    i_s = ip.tile([128, 64], FP32R, tag="is", name="is")
                    nc.sync.dma_start(i_s[:], inds.ap()[:, mb * 64:(mb + 1) * 64])
                    i_q = ip.tile([128, 64], FP32R, tag="iq", name="iq")
                    nc.sync.dma_start(i_q[:], indq.ap()[:, mb * 64:(mb + 1) * 64])
                    for n in range(NTC):
                        nsl = slice(n * 512, (n + 1) * 512)
                        mmo = ps.tile([128, 512], FP32, tag="mmo", name="mmo")
                        for h2 in range(2):
                            osl = slice(h2 * 256, (h2 + 1) * 256)
                            for j in range(NJH):
                                nc.tensor.matmul(mmo[:, osl],
                                                 wqkt[j][:, :, mb * 128:(mb + 1) * 128],
                                                 xn8[n][:, 2 * j:2 * j + 2, osl],
                                                 start=(h2 == 0 and j == 0),
                                                 stop=(h2 == 1 and j == NJH - 1),
                                                 perf_mode=DR, skip_group_check=True)
                        qs = sb.tile([128, 512], FP32R, tag="qs", name="qs")
                        nc.scalar.activation(qs[:], mmo[:], AF.Identity,
                                             bias=bqk_s[:, mb:mb + 1], scale=ISCL)
                        sq = sb.tile([128, 512], FP32R, tag="sq", name="sq")
                        nc.vector.tensor_mul(sq[:], qs[:], qs[:])
                        nc.tensor.matmul(qstat[n][:], i_s[:], qs[:],
                                         start=(mb == 0), stop=False, skip_group_check=True)
                        nc.tensor.matmul(qstat[n][:], i_q[:], sq[:],
                                         start=False, stop=(mb == QKF - 1), skip_group_check=True)
                        nc.sync.dma_start(qk_d[mb * 128:(mb + 1) * 128, nsl], qs[:])
                # stats psum rows: means (qsum 0-15, ksum 16-31), sq-means (32-63)
                for n in range(NTC):
                    nsl = slice(n * 512, (n + 1) * 512)
                    ms64 = sb.tile([64, 512], FP32, tag="ms64", name="ms64")
                    nc.scalar.mul(ms64[:], qstat[n][:], 1.0 / HD)
                    msq = sb.tile([32, 512], FP32, tag="msqh", name="msqh")
                    nc.sync.dma_start(msq[:], ms64[32:64, :])
                    m2 = sb.tile([32, 512], FP32, tag="m2h", name="m2h")
                    nc.vector.tensor_mul(m2[:], ms64[0:32, :], ms64[0:32, :])
                    var = sb.tile([32, 512], FP32, tag="varh", name="varh")
                    nc.vector.tensor_sub(var[:], msq[:], m2[:])
                    sd = sb.tile([32, 512], FP32, tag="sdh", name="sdh")
                    nc.scalar.activation(sd[:], var[:], AF.Sqrt, bias=eps_col[0:32, :], scale=1.0)
                    nc.vector.reciprocal(stq_r[:, nsl], sd[:])
                    nc.vector.tensor_mul(stq_mr[:, nsl], ms64[0:32, :], stq_r[:, nsl])

        # qkv v-part: xn8 stationary -> v token-major (fp8 DoubleRow)
        if PHASES >= 4:
            with ExitStack() as ph:
                sb = ph.enter_context(tc.tile_pool(name="vsb", bufs=3))
                wp = ph.enter_context(tc.tile_pool(name="vw", bufs=1))
                ps = ph.enter_context(tc.tile_pool(name="vps", bufs=2, space="PSUM"))
                wvt = [wp.tile([128, 2, H], FP8, tag=f"wv{j}", name=f"wv{j}")
                       for j in range(NJH)]
                for j in range(NJH):
                    nc.sync.dma_start(wvt[j][:], wv8.ap()[:, j * 2 * H:(j + 1) * 2 * H])
                VCW = [(0, 256), (256, 256), (512, 256), (768, 256), (1024, 128)]
                for tb in range(T // 128):
                    n, tc0 = tb // 4, (tb % 4) * 128
                    tsl = slice(tb * 128, (tb + 1) * 128)
                    for (v0, vw) in VCW:
                        vp = ps.tile([128, 256], FP32, tag="vp", name="vp")
                        for j in range(NJH):
                            nc.tensor.matmul(vp[:, 0:vw],
                                             xn8[n][:, 2 * j:2 * j + 2, tc0:tc0 + 128],
                                             wvt[j][:, :, v0:v0 + vw],
                                             start=(j == 0), stop=False,
                                             perf_mode=DR, skip_group_check=True)
                        nc.tensor.matmul(vp[:, 0:vw], ones512_8[0:1, 0:128],
                                         bv8_s[0:1, v0:v0 + vw],
                                         start=False, stop=True, skip_group_check=True)
                        vs = sb.tile([128, 256], FP8, tag="vs", name="vs")
                        nc.vector.tensor_scalar_mul(vs[:, 0:vw], vp[:, 0:vw], ISCL)
                        nc.sync.dma_start(v_d[tsl, v0:v0 + vw], vs[:, 0:vw])

        if DBG != "xn":
            xn_p.release()

        # ---------------- phase 2: attention ----------------
        if PHASES >= 5:
            with ExitStack() as ph:
                ih = ph.enter_context(tc.tile_pool(name="ihp", bufs=1))
                indh_s = ih.tile([32, 32 * HD], FP32R, tag="indh", name="indh")
                nc.sync.dma_start(indh_s[:], indh.ap())
                qp = ph.enter_context(tc.tile_pool(name="aq", bufs=2))
                up = ph.enter_context(tc.tile_pool(name="au", bufs=2))
                vpl = ph.enter_context(tc.tile_pool(name="av", bufs=2))
                ob = ph.enter_context(tc.tile_pool(name="ao", bufs=2))
                pse = ph.enter_context(tc.tile_pool(name="pse", bufs=1, space="PSUM"))
                pss = ph.enter_context(tc.tile_pool(name="pss", bufs=2, space="PSUM"))
                pso = ph.enter_context(tc.tile_pool(name="pso", bufs=2, space="PSUM"))
                psz = ph.enter_context(tc.tile_pool(name="psz", bufs=1, space="PSUM"))
                for b in range(BPC):
                    c0 = b * N
                    for h in range(NH):
                        r0 = h * HD
                        qr = qp.tile([HD, N], FP32R, tag="qr", name="qr")
                        nc.sync.dma_start(qr[:], qk_d[r0:r0 + HD, c0:c0 + N])
                        kr = qp.tile([HD, N], FP32R, tag="kr", name="kr")
                        nc.sync.dma_start(kr[:], qk_d[H + r0:H + r0 + HD, c0:c0 + N])
                        qn = qp.tile([HD, N], FP32R, tag="qn", name="qn")
                        kn = qp.tile([HD, N], FP32R, tag="kn", name="kn")
                        for q2 in range(2):
                            lsl = slice(q2 * 512, (q2 + 1) * 512)
                            gsl = slice(c0 + q2 * 512, c0 + q2 * 512 + 512)
                            for (dst, src, gi0) in ((qn, qr, 0), (kn, kr, 16)):
                                rp = pse.tile([HD, 512], FP32, tag="rp", name="rp")
                                nc.tensor.matmul(rp[:], indh_s[:, (gi0 + h) * HD:(gi0 + h + 1) * HD],
                                                 stq_r[:, gsl], start=True, stop=True)
                                mp = pse.tile([HD, 512], FP32, tag="mp", name="mp")
                                nc.tensor.matmul(mp[:], indh_s[:, (gi0 + h) * HD:(gi0 + h + 1) * HD],
                                                 stq_mr[:, gsl], start=True, stop=True)
                                tq = qp.tile([HD, 512], FP32, tag="tq", name="tq")
                                nc.vector.tensor_tensor(tq[:], src[:, lsl], rp[:], op=ALU.mult)
                                nc.vector.tensor_tensor(dst[:, lsl], tq[:], mp[:], op=ALU.subtract)
                        vL = []
                        for p in range(4):  # ktok chunk pairs for DoubleRow
                            vt = vpl.tile([128, 2, 128], FP8, tag=f"vL{p}", name=f"vL{p}")
                            for i in range(2):
                                t0 = c0 + (2 * p + i) * 128
                                nc.sync.dma_start(vt[:, i:i + 1, 0:HD],
                                                  v_d[t0:t0 + 128, r0:r0 + HD])
                            nc.vector.memset(vt[:, :, HD:96], 0.0)
                            nc.vector.memset(vt[:, :, 96:97], 1.0)
                            nc.vector.memset(vt[:, :, 97:128], 0.0)
                            vL.append(vt)
                        # u = exp(z - 2) in fp8 (e4m3 max 448; z <= ~6 so safe)
                        ut = up.tile([128, 8, N], FP8, tag="ut", name="ut")
                        for nk in range(8):
                            for q2 in range(2):
                                lsl = slice(q2 * 512, (q2 + 1) * 512)
                                sp = pss.tile([128, 512], FP32, tag="sp", name="sp")
                                nc.tensor.matmul(sp[:], kn[:, nk * 128:(nk + 1) * 128],
                                                 qn[:, lsl], start=True, stop=True)
                                nc.scalar.activation(ut[:, nk:nk + 1, lsl], sp[:], AF.Exp,
                                                     bias=neg2[:], scale=ISQ)
                        for q2 in range(2):
                            op = pso.tile([128, 512], FP32, tag="op", name="op")
                            for c2 in range(2):
                                csl = slice(q2 * 512 + c2 * 256, q2 * 512 + c2 * 256 + 256)
                                for p in range(4):
                                    nc.tensor.matmul(op[:, c2 * 256:c2 * 256 + 256],
                                                     vL[p][:], ut[:, 2 * p:2 * p + 2, csl],
                                                     start=(c2 == 0 and p == 0),
                                                     stop=(c2 == 1 and p == 3),
                                                     perf_mode=DR, skip_group_check=True)
                            osb = ob.tile([HD, 512], FP32R, tag="osb", name="osb")
                            nc.scalar.copy(osb[:], op[0:HD, :])
                            rz97 = ob.tile([128, 512], FP32R, tag="rz97", name="rz97")
                            nc.vector.reciprocal(rz97[96:97, :], op[96:97, :])
                            rz0 = ob.tile([1, 512], FP32R, tag="rz0", name="rz0")
                            nc.sync.dma_start(rz0[:], rz97[96:97, :])
                            rzp = psz.tile([HD, 512], FP32, tag="rzp", name="rzp")
                            nc.tensor.matmul(rzp[:], ones72, rz0[:], start=True, stop=True)
                            o2 = ob.tile([HD, 512], FP8, tag="o2", name="o2")
                            nc.vector.tensor_tensor(o2[:], osb[:], rzp[:], op=ALU.mult)
                            nc.sync.dma_start(o_d[r0:r0 + HD, c0 + q2 * 512:c0 + q2 * 512 + 512],
                                              o2[:])

        stq_p.release()

        # ---------------- phase 3: proj + gated residual -> x2 ----------------
        if PHASES >= 6:
            with ExitStack() as ph:
                opl = ph.enter_context(tc.tile_pool(name="po", bufs=1))
                wp = ph.enter_context(tc.tile_pool(name="pw", bufs=1))
                sb = ph.enter_context(tc.tile_pool(name="psb", bufs=3))
                ps = ph.enter_context(tc.tile_pool(name="pps", bufs=2, space="PSUM"))
                oc8 = [opl.tile([128, NCH, 512], FP8, tag=f"oc{n}", name=f"oc{n}")
                       for n in range(NTC)]
                for n in range(NTC):
                    nc.vector.memset(oc8[n][:, FK:NCH, :], 0.0)
                    nsl = slice(n * 512, (n + 1) * 512)
                    for ft in range(FK):
                        nc.sync.dma_start(oc8[n][:, ft:ft + 1, :],
                                          o_d[ft * 128:(ft + 1) * 128, nsl])
                wprot = [wp.tile([128, 2, H], FP8, tag=f"wp{j}", name=f"wp{j}")
                         for j in range(NJH)]
                for j in range(NJH):
                    nc.sync.dma_start(wprot[j][:], wpro8.ap()[:, j * 2 * H:(j + 1) * 2 * H])
                for mb in range(FK):
                    msl = slice(mb * 128, (mb + 1) * 128)
                    for n in range(NTC):
                        b = n // (NTC // BPC)
                        nsl = slice(n * 512, (n + 1) * 512)
                        mmo = ps.tile([128, 512], FP32, tag="mmo", name="mmo")
                        for h2 in range(2):
                            osl = slice(h2 * 256, (h2 + 1) * 256)
                            for j in range(NJH):
                                nc.tensor.matmul(mmo[:, osl],
                                                 wprot[j][:, :, msl],
                                                 oc8[n][:, 2 * j:2 * j + 2, osl],
                                                 start=(h2 == 0 and j == 0), stop=False,
                                                 perf_mode=DR, skip_group_check=True)
                            nc.tensor.matmul(mmo[:, osl], bpro8_s[0:1, msl],
                                             ones512_8[0:1, 0:256],
                                             start=False, stop=(h2 == 1),
                                             skip_group_check=True)
                        xr = sb.tile([128, 512], FP32R, tag="xr", name="xr")
                        nc.sync.dma_start(xr[:], xT.ap()[msl, nsl])
                        x2s = sb.tile([128, 512], FP32R, tag="x2s", name="x2s")
                        nc.vector.scalar_tensor_tensor(x2s[:], mmo[:], gcol16[0][b][:, mb:mb + 1],
                                                       xr[:], op0=ALU.mult, op1=ALU.add)
                        nc.sync.dma_start(x2_d[msl, nsl], x2s[:])

        # ---------------- phase 4+5: LN2 -> xn2; fused MLP ----------------
        if PHASES >= 7:
            xn2_p = tc.alloc_tile_pool(name="xn2p", bufs=1)
            xn2_8 = [xn2_p.tile([128, NCH, 512], FP8, tag=f"xn2{n}", name=f"xn2{n}")
                     for n in range(NTC)]
            _DBG_REFS["xn2"] = xn2_8
            for n in range(NTC):
                nc.vector.memset(xn2_8[n][:, FK:NCH, :], 0.0)
            ln_modulate(x2_d, 1, xn2_8)

            with ExitStack() as ph:
                w1p_ = ph.enter_context(tc.tile_pool(name="m1w", bufs=1))
                w2p_ = ph.enter_context(tc.tile_pool(name="m2w", bufs=1))
                hp = ph.enter_context(tc.tile_pool(name="mhp", bufs=2))
                sb = ph.enter_context(tc.tile_pool(name="msb", bufs=3))
                ps = ph.enter_context(tc.tile_pool(name="mps", bufs=2, space="PSUM"))
                w1t = [w1p_.tile([128, 2, MLP], FP8, tag=f"w1{j}", name=f"w1{j}")
                       for j in range(NJH)]
                for j in range(NJH):
                    nc.sync.dma_start(w1t[j][:], w18.ap()[:, j * 2 * MLP:(j + 1) * 2 * MLP])
                w2t = [w2p_.tile([128, 2, H], FP8, tag=f"w2{j}", name=f"w2{j}")
                       for j in range(NJM)]
                for j in range(NJM):
                    nc.sync.dma_start(w2t[j][:], w28.ap()[:, j * 2 * H:(j + 1) * 2 * H])
                for n in range(NTC):
                    b = n // (NTC // BPC)
                    nsl = slice(n * 512, (n + 1) * 512)
                    h8n = hp.tile([128, MK, 512], FP8, tag="h8", name="h8")
                    for mb in range(MK):
                        mmo = ps.tile([128, 512], FP32, tag="mmo", name="mmo")
                        for h2 in range(2):
                            osl = slice(h2 * 256, (h2 + 1) * 256)
                            for j in range(NJH):
                                nc.tensor.matmul(mmo[:, osl],
                                                 w1t[j][:, :, mb * 128:(mb + 1) * 128],
                                                 xn2_8[n][:, 2 * j:2 * j + 2, osl],
                                                 start=(h2 == 0 and j == 0),
                                                 stop=(h2 == 1 and j == NJH - 1),
                                                 perf_mode=DR, skip_group_check=True)
                        nc.scalar.activation(h8n[:, mb:mb + 1, :], mmo[:], AF.Gelu_apprx_tanh,
                                             bias=b1_s[:, mb:mb + 1], scale=ISCL)
                    for mb in range(FK):
                        msl = slice(mb * 128, (mb + 1) * 128)
                        mm2 = ps.tile([128, 512], FP32, tag="mm2", name="mm2")
                        for h2 in range(2):
                            osl = slice(h2 * 256, (h2 + 1) * 256)
                            for j in range(NJM):
                                nc.tensor.matmul(mm2[:, osl],
                                                 w2t[j][:, :, msl],
                                                 h8n[:, 2 * j:2 * j + 2, osl],
                                                 start=(h2 == 0 and j == 0), stop=False,
                                                 perf_mode=DR, skip_group_check=True)
                            nc.tensor.matmul(mm2[:, osl], b28_s[0:1, msl],
                                             ones512_8[0:1, 0:256],
                                             start=False, stop=(h2 == 1),
                                             skip_group_check=True)
                        x2r = sb.tile([128, 512], FP32R, tag="x2r", name="x2r")
                        nc.sync.dma_start(x2r[:], x2_d[msl, nsl])
                        os_ = sb.tile([128, 512], FP32, tag="os", name="os")
                        nc.vector.scalar_tensor_tensor(os_[:], mm2[:], gcol16[1][b][:, mb:mb + 1],
                                                       x2r[:], op0=ALU.mult, op1=ALU.add)
                        nc.sync.dma_start(out.ap()[msl, nsl], os_[:])
            if DBG != "xn2":
                xn2_p.release()

        # ---------------- debug dumps ----------------
        if DBG is not None:
            with ExitStack() as ph:
                db = ph.enter_context(tc.tile_pool(name="dbg", bufs=2))
                if DBG == "mod":
                    dtile = db.tile([128, 108], FP32, tag="dmp", name="dmp")
                    nc.vector.tensor_copy(dtile[:], _DBG_REFS["modc"][:])
                    nc.sync.dma_start(out.ap()[0:128, 0:108], dtile[:])
                elif DBG in ("xn", "xn2"):
                    src = _DBG_REFS[DBG]
                    for n in range(NTC):
                        nsl = slice(n * 512, (n + 1) * 512)
                        for ft in range(FK):
                            dtile = db.tile([128, 512], FP32, tag="dmp", name="dmp")
                            nc.vector.tensor_copy(dtile[:], src[n][:, ft:ft + 1, :])
                            nc.sync.dma_start(out.ap()[ft * 128:(ft + 1) * 128, nsl],
                                              dtile[:])
                elif DBG == "qk":
                    for ft in range(FK):  # dump q rows only
                        nc.sync.dma_start(out.ap()[ft * 128:(ft + 1) * 128, :],
                                          qk_d[ft * 128:(ft + 1) * 128, :].bitcast(FP32))
                elif DBG == "v":
                    for tb in range(T // 128):  # v_d [T, H] -> out[0:H? dump transposed blocks
                        pass
                elif DBG == "o":
                    for ft in range(FK):
                        for n in range(NTC):
                            nsl = slice(n * 512, (n + 1) * 512)
                            stile = db.tile([128, 512], FP8, tag="dm8", name="dm8")
                            nc.sync.dma_start(stile[:], o_d[ft * 128:(ft + 1) * 128, nsl])
                            dtile = db.tile([128, 512], FP32, tag="dmp", name="dmp")
                            nc.vector.tensor_copy(dtile[:], stile[:])
                            nc.sync.dma_start(out.ap()[ft * 128:(ft + 1) * 128, nsl],
                                              dtile[:])
                elif DBG == "x2":
                    for ft in range(FK):
                        nc.sync.dma_start(out.ap()[ft * 128:(ft + 1) * 128, :],
                                          x2_d[ft * 128:(ft + 1) * 128, :].bitcast(FP32))

    nc.finalize()
    return nc


def _pack_dr(w):
    """[Kin, Cols] fp32 -> fp8 [128, njp*2*Cols]: packed[p, (j, i, c)] =
    w[(2j+i)*128 + p, c], zero-padded along Kin to a multiple of 256."""
    kin, cols = w.shape
    njp = -(-kin // 256)
    wp = np.zeros((njp * 256, cols), np.float32)
    wp[:kin] = w
    t = wp.reshape(njp, 2, 128, cols).transpose(2, 0, 1, 3)  # [p, j, i, c]
    return np.ascontiguousarray(
        t.reshape(128, njp * 2 * cols).astype(ml_dtypes.float8_e4m3))


def _host_inputs(x, c, w_mod, b_mod, w_qkv, b_qkv, g_q, g_k, w_proj, b_proj,
                 w1, b1, w2, b2):
    f32 = np.float32
    bf16 = ml_dtypes.bfloat16
    fp8 = ml_dtypes.float8_e4m3
    w_qkv = np.asarray(w_qkv, f32)

    # bmodc[p, c] = b_mod[c*128+p] (+1 for the (1+sc) chunks: quantities 1, 4)
    bm = np.asarray(b_mod, f32).reshape(54, 128).T.copy()
    for q in (1, 4):
        bm[:, q * 9:(q + 1) * 9] += 1.0

    shared = {
        "wmodb": np.ascontiguousarray(np.asarray(w_mod, f32)).astype(bf16),
        "bmodc": np.ascontiguousarray(bm),
        "wqk8": _pack_dr(w_qkv[:, :2 * H] * SCL),
        "wv8": _pack_dr(w_qkv[:, 2 * H:] * SCL),
        "bqk": np.ascontiguousarray(np.asarray(b_qkv, f32)[:2 * H].reshape(QKF, 128).T),
        "bv8": (np.asarray(b_qkv, f32)[2 * H:] * SCL).reshape(1, H).astype(fp8),
        "wpro8": _pack_dr(np.asarray(w_proj, f32) * SCL),
        "bpro8": (np.asarray(b_proj, f32) * SCL).reshape(1, H).astype(fp8),
        "w18": _pack_dr(np.asarray(w1, f32) * SCL),
        "b1c": np.ascontiguousarray(np.asarray(b1, f32).reshape(MK, 128).T),
        "w28": _pack_dr(np.asarray(w2, f32) * SCL),
        "b28": (np.asarray(b2, f32) * SCL).reshape(1, H).astype(fp8),
    }

    # stat rows: qsum=h, ksum=16+h, qsqsum=32+h, ksqsum=48+h
    ind_s = np.zeros((128, QKF * 64), f32)
    ind_q = np.zeros((128, QKF * 64), f32)
    for mb in range(QKF):
        for f in range(128):
            gf = mb * 128 + f
            if gf < H:
                hh, base = gf // HD, 0
            else:
                hh, base = (gf - H) // HD, 16
            ind_s[f, mb * 64 + base + hh] = 1.0
            ind_q[f, mb * 64 + base + 32 + hh] = 1.0
    shared["inds"] = ind_s
    shared["indq"] = ind_q

    # expand rows (stq_r / stq_mr): rq=h, rk=16+h; one [32,72] block per head-slot
    ih = np.zeros((32, 32 * HD), f32)
    for s in range(32):
        gvec = g_q if s < 16 else g_k
        ih[s, s * HD:(s + 1) * HD] = np.asarray(gvec, f32)
    shared["indh"] = ih
    shared["onesr"] = np.ones((1, 512), f32)
    shared["ones8r"] = np.ones((1, 512), f32).astype(fp8)
    shared["onesc"] = np.ones((128, 1), f32)
    shared["epsc"] = np.full((128, 1), EPS, f32)
    vt = np.zeros((128, 25), f32)
    vt[:, 24] = 1.0
    shared["vtail"] = vt

    in_maps = []
    for core in range(NCORES):
        xs = np.asarray(x[core * BPC:(core + 1) * BPC], f32)   # [2, N, H]
        m = dict(shared)
        m["xT"] = np.ascontiguousarray(xs.reshape(T, H).T)
        m["cT"] = np.ascontiguousarray(np.asarray(c[core * BPC:(core + 1) * BPC], f32).T)
        in_maps.append(m)
    return in_maps


def kernel(**inputs):
    if "nc" not in _CACHE:
        _CACHE["nc"] = _build_program()
    nc = _CACHE["nc"]
    in_maps = _host_inputs(**inputs)
    res = run_bass_kernel_spmd(nc, in_maps, core_ids=list(range(NCORES)))
    outs = [res.results[core]["out"].T.reshape(BPC, N, H) for core in range(NCORES)]
    return np.concatenate(outs, axis=0).astype(np.float32)


# revision 25
# speedup vs baseline: 1.0615x; 1.0615x over previous
"""DiT block kernel for 8 Trainium2 NeuronCores.

Strategy: pure data-parallel over batch (16 batches -> 2 per core), activations
feature-major ([feature, token]). Big GEMMs (qkv, v, proj, mlp1, mlp2) run as
fp8e4 DoubleRow matmuls (0.5 cycles/row, 256-deep contraction) with weights
host-scaled by 16 to dodge fp8 subnormals; the 1/16 is folded into the PSUM
evacuation (Act scale or scalar_tensor_tensor scalar). Contraction dims are
zero-padded to a multiple of 256 (H: 9 chunks -> 10). The adaLN modulation is
computed feature-major in bf16 (tiny output, gates need precision), giving
per-partition scale/bias columns for free. LayerNorm / softmax reductions run
as ones/indicator matmuls on the PE (fp32r); per-token stats broadcast back via
K=1 expansion matmuls. Softmax normalization is deferred through attn@v via an
appended ones-column on v. MLP1->MLP2 are fused per 512-token block (hidden
stays in SBUF as fp8).
"""

import sys

sys.path.insert(0, "/opt/trn_rl_repo")

from contextlib import ExitStack

import ml_dtypes
import numpy as np

import concourse.bacc as bacc
import concourse.tile as tile
from concourse import mybir
from concourse.bass_utils import run_bass_kernel_spmd

FP32 = mybir.dt.float32
FP32R = mybir.dt.float32r
BF16 = mybir.dt.bfloat16
FP8 = mybir.dt.float8e4
AF = mybir.ActivationFunctionType
ALU = mybir.AluOpType
DR = mybir.MatmulPerfMode.DoubleRow

B, N, H = 16, 1024, 1152
NH, HD = 16, 72
MLP = H * 4
NCORES = 8
BPC = B // NCORES            # batches per core = 2
T = BPC * N                  # tokens per core = 2048
NTC = T // 512               # token chunks of 512 = 4
FK = H // 128                # 9
NCH = FK + 1                 # padded contraction chunks for H = 10
NJH = NCH // 2               # DoubleRow pair-steps for H = 5
MK = MLP // 128              # 36
NJM = MK // 2                # DoubleRow pair-steps for MLP = 18
QKF = (2 * H) // 128         # 18 qk feature tiles
EPS = 1e-6
ISQ = float(HD) ** -0.5
SCL = 16.0                   # fp8 weight scale
ISCL = 1.0 / SCL

_CACHE = {}
_DBG_REFS = {}
PHASES = 8  # build phases up to this number (bisect aid)
DBG = None  # None | "mod" | "xn" | "qk" | "v" | "o" | "x2" | "xn2": dump intermediate to out


def _build_program():
    nc = bacc.Bacc("TRN2", target_bir_lowering=False, debug=False)

    xT = nc.dram_tensor("xT", [H, T], FP32R, kind="ExternalInput")
    cT = nc.dram_tensor("cT", [H, BPC], FP32, kind="ExternalInput")
    wmodb = nc.dram_tensor("wmodb", [H, 6 * H], BF16, kind="ExternalInput")
    bmodc = nc.dram_tensor("bmodc", [128, 54], FP32, kind="ExternalInput")
    wqk8 = nc.dram_tensor("wqk8", [128, NJH * 2 * 2 * H], FP8, kind="ExternalInput")
    wv8 = nc.dram_tensor("wv8", [128, NJH * 2 * H], FP8, kind="ExternalInput")
    bqk = nc.dram_tensor("bqk", [128, QKF], FP32, kind="ExternalInput")
    bv8 = nc.dram_tensor("bv8", [1, H], FP8, kind="ExternalInput")
    wpro8 = nc.dram_tensor("wpro8", [128, NJH * 2 * H], FP8, kind="ExternalInput")
    bpro8 = nc.dram_tensor("bpro8", [1, H], FP8, kind="ExternalInput")
    w18 = nc.dram_tensor("w18", [128, NJH * 2 * MLP], FP8, kind="ExternalInput")
    b1c = nc.dram_tensor("b1c", [128, MK], FP32, kind="ExternalInput")
    w28 = nc.dram_tensor("w28", [128, NJM * 2 * H], FP8, kind="ExternalInput")
    b28 = nc.dram_tensor("b28", [1, H], FP8, kind="ExternalInput")
    inds = nc.dram_tensor("inds", [128, QKF * 64], BF16, kind="ExternalInput")
    indq = nc.dram_tensor("indq", [128, QKF * 64], BF16, kind="ExternalInput")
    indh = nc.dram_tensor("indh", [32, 32 * HD], FP32R, kind="ExternalInput")
    onesr = nc.dram_tensor("onesr", [1, 512], FP32R, kind="ExternalInput")
    ones8r = nc.dram_tensor("ones8r", [1, 512], FP8, kind="ExternalInput")
    onesc = nc.dram_tensor("onesc", [128, 1], FP32R, kind="ExternalInput")
    epsc = nc.dram_tensor("epsc", [128, 1], FP32, kind="ExternalInput")
    vtail = nc.dram_tensor("vtail", [128, 25], FP32R, kind="ExternalInput")
    out = nc.dram_tensor("out", [H, T], FP32, kind="ExternalOutput")

    with nc.allow_low_precision(
        reason="fp8 DoubleRow matmuls with x16 weight scaling; errors diluted "
        "by the residual stream, gate 2e-2 has big margin"
    ), tile.TileContext(nc) as tc, ExitStack() as top:
        dram = top.enter_context(tc.tile_pool(name="dram", bufs=1, space="DRAM"))
        qk_d = dram.tile([2 * H, T], BF16, tag="qk_d", name="qk_d")
        v_d = dram.tile([T, H], FP8, tag="v_d", name="v_d")
        o_d = dram.tile([H, T], FP8, tag="o_d", name="o_d")
        x2_d = dram.tile([H, T], BF16, tag="x2_d", name="x2_d")

        cst = top.enter_context(tc.tile_pool(name="cst", bufs=1))
        ones512 = cst.tile([1, 512], FP32R, tag="o512", name="o512")
        nc.sync.dma_start(ones512[:], onesr.ap())
        ones512_8 = cst.tile([1, 512], FP8, tag="o512f8", name="o512f8")
        nc.sync.dma_start(ones512_8[:], ones8r.ap())
        ocol128 = cst.tile([128, 1], FP32R, tag="oc128", name="oc128")
        nc.sync.dma_start(ocol128[:], onesc.ap())
        eps_col = cst.tile([128, 1], FP32, tag="epsc", name="epsc")
        nc.sync.dma_start(eps_col[:], epsc.ap())
        vtail_s = cst.tile([128, 25], FP32R, tag="vtl", name="vtl")
        nc.sync.dma_start(vtail_s[:], vtail.ap())
        neg2 = cst.tile([128, 1], FP32, tag="neg2", name="neg2")
        nc.vector.memset(neg2[:], -2.0)
        ocol_b = cst.tile([128, 1], BF16, tag="oc128b", name="oc128b")
        nc.vector.memset(ocol_b[:], 1.0)
        ones1x1 = ones512[0:1, 0:1]
        ones72 = ones512[0:1, 0:HD]

        bqk_s = cst.tile([128, QKF], FP32, tag="bqk", name="bqk")
        nc.sync.dma_start(bqk_s[:], bqk.ap())
        bv8_s = cst.tile([1, H], FP8, tag="bv8", name="bv8")
        nc.sync.dma_start(bv8_s[:], bv8.ap())
        bpro8_s = cst.tile([1, H], FP8, tag="bpro8", name="bpro8")
        nc.sync.dma_start(bpro8_s[:], bpro8.ap())
        b1_s = cst.tile([128, MK], FP32, tag="b1", name="b1")
        nc.sync.dma_start(b1_s[:], b1c.ap())
        b28_s = cst.tile([1, H], FP8, tag="b28", name="b28")
        nc.sync.dma_start(b28_s[:], b28.ap())
        bmodc_s = cst.tile([128, 54], FP32, tag="bmodc", name="bmodc")
        nc.sync.dma_start(bmodc_s[:], bmodc.ap())

        gcol_p = top.enter_context(tc.tile_pool(name="gcolp", bufs=1))
        gcol16 = [[gcol_p.tile([128, FK], FP32, tag=f"gc{u}{b}", name=f"gc{u}{b}")
                   for b in range(BPC)] for u in range(2)]

        mod_p = top.enter_context(tc.tile_pool(name="modp", bufs=1))
        sc_t = [[mod_p.tile([1, H], FP32R, tag=f"sc{u}{b}", name=f"sc{u}{b}") for b in range(BPC)]
                for u in range(2)]
        sh_t = [[mod_p.tile([1, H], FP32R, tag=f"sh{u}{b}", name=f"sh{u}{b}") for b in range(BPC)]
                for u in range(2)]

        stq_p = tc.alloc_tile_pool(name="stqp", bufs=1)
        stq_r = stq_p.tile([32, T], FP32R, tag="stq_r", name="stq_r")
        stq_mr = stq_p.tile([32, T], FP32R, tag="stq_mr", name="stq_mr")

        # ---------------- phase 0: adaLN modulation (feature-major, bf16) ----
        with ExitStack() as ph:
            sb = ph.enter_context(tc.tile_pool(name="p0sb", bufs=2))
            wm = ph.enter_context(tc.tile_pool(name="p0wm", bufs=2))
            ps = ph.enter_context(tc.tile_pool(name="p0ps", bufs=1, space="PSUM"))
            modc = mod_p.tile([128, 108], FP32, tag="modc", name="modc")
            _DBG_REFS["modc"] = modc
            swcb = mod_p.tile([128, 2 * FK], BF16, tag="swcb", name="swcb")
            for k in range(FK):
                craw = sb.tile([128, BPC], FP32, tag="craw", name="craw")
                nc.sync.dma_start(craw[:], cT.ap()[k * 128:(k + 1) * 128, :])
                nc.scalar.activation(swcb[:, 2 * k:2 * k + 2], craw[:], AF.Silu,
                                     bias=0.0, scale=1.0)
            pm = ps.tile([128, 108], FP32, tag="pm", name="pm")
            for k in range(FK):
                wmk = wm.tile([128, 6 * H], BF16, tag="wmk", name="wmk")
                nc.sync.dma_start(wmk[:], wmodb.ap()[k * 128:(k + 1) * 128, :])
                for c in range(54):
                    # one accumulation group for the whole 2KB psum region:
                    # start only zeroes once (it clears the full bank row)
                    nc.tensor.matmul(pm[:, 2 * c:2 * c + 2],
                                     wmk[:, c * 128:(c + 1) * 128],
                                     swcb[:, 2 * k:2 * k + 2],
                                     start=(k == 0 and c == 0),
                                     stop=(k == FK - 1 and c == 53),
                                     skip_group_check=True)
            for c in range(54):
                nc.scalar.activation(modc[:, 2 * c:2 * c + 2], pm[:, 2 * c:2 * c + 2],
                                     AF.Identity, bias=bmodc_s[:, c:c + 1], scale=1.0)
            # columns -> row vectors (sc, sh) and gate columns /16
            for u in range(2):
                for b in range(BPC):
                    for ft in range(FK):
                        csc = ((1 + 3 * u) * FK + ft) * 2 + b
                        csh = ((0 + 3 * u) * FK + ft) * 2 + b
                        cg = ((2 + 3 * u) * FK + ft) * 2 + b
                        fsl = slice(ft * 128, (ft + 1) * 128)
                        nc.sync.dma_start(sc_t[u][b][0:1, fsl],
                                          modc[:, csc:csc + 1].bitcast(FP32R))
                        nc.sync.dma_start(sh_t[u][b][0:1, fsl],
                                          modc[:, csh:csh + 1].bitcast(FP32R))
                        nc.vector.tensor_scalar_mul(gcol16[u][b][:, ft:ft + 1],
                                                    modc[:, cg:cg + 1], ISCL)

        # ---------------- LayerNorm + modulation -> fp8 chunk-major ----------
        def ln_modulate(src_dram, u, dst_tiles, src_dt=FP32R, ones_col=None):
            if ones_col is None:
                ones_col = ocol128
            with ExitStack() as ph:
                sb = ph.enter_context(tc.tile_pool(name="lnsb", bufs=1))
                xp = ph.enter_context(tc.tile_pool(name="lnx", bufs=1))
                ps = ph.enter_context(tc.tile_pool(name="lnps", bufs=2, space="PSUM"))
                pe = ph.enter_context(tc.tile_pool(name="lnpe", bufs=2, space="PSUM"))
                for n in range(NTC):
                    b = n // (NTC // BPC)
                    nsl = slice(n * 512, (n + 1) * 512)
                    xc = [xp.tile([128, 512], src_dt, tag=f"xc{k}", name=f"xc{k}") for k in range(FK)]
                    ln_s = ps.tile([1, 512], FP32, tag="lns", name="lns")
                    ln_q = ps.tile([1, 512], FP32, tag="lnq", name="lnq")
                    for k in range(FK):
                        nc.gpsimd.dma_start(xc[k][:], src_dram[k * 128:(k + 1) * 128, nsl])
                        sq = sb.tile([128, 512], src_dt, tag="sq", name="sq", bufs=3)
                        nc.vector.tensor_mul(sq[:], xc[k][:], xc[k][:])
                        nc.tensor.matmul(ln_s[:], ones_col[:], xc[k][:],
                                         start=(k == 0), stop=(k == FK - 1))
                        nc.tensor.matmul(ln_q[:], ones_col[:], sq[:],
                                         start=(k == 0), stop=(k == FK - 1))
                    ms_s = sb.tile([1, 512], FP32, tag="ms_s", name="ms_s")
                    nc.scalar.mul(ms_s[:], ln_s[:], 1.0 / H)
                    ms_q = sb.tile([1, 512], FP32, tag="ms_q", name="ms_q")
                    nc.scalar.mul(ms_q[:], ln_q[:], 1.0 / H)
                    m2 = sb.tile([1, 512], FP32, tag="m2", name="m2")
                    nc.vector.tensor_mul(m2[:], ms_s[:], ms_s[:])
                    var = sb.tile([1, 512], FP32, tag="var", name="var")
                    nc.vector.tensor_sub(var[:], ms_q[:], m2[:])
                    sd = sb.tile([1, 512], FP32, tag="sd", name="sd")
                    nc.scalar.activation(sd[:], var[:], AF.Sqrt, bias=eps_col[0:1, :], scale=1.0)
                    stA = sb.tile([1, 512], FP32R, tag="stA", name="stA")
                    nc.vector.reciprocal(stA[:], sd[:])
                    stC = sb.tile([1, 512], FP32R, tag="stC", name="stC")
                    mA = sb.tile([1, 512], FP32, tag="mA", name="mA")
                    nc.vector.tensor_mul(mA[:], ms_s[:], stA[:])
                    nc.vector.tensor_scalar_mul(stC[:], mA[:], -1.0)
                    for ft in range(FK):
                        fsl = slice(ft * 128, (ft + 1) * 128)
                        al = pe.tile([128, 512], FP32, tag="al", name="al")
                        nc.tensor.matmul(al[:], sc_t[u][b][0:1, fsl], stA[:],
                                         start=True, stop=True)
                        be = pe.tile([128, 512], FP32, tag="be", name="be")
                        nc.tensor.matmul(be[:], sc_t[u][b][0:1, fsl], stC[:],
                                         start=True, stop=False)
                        nc.tensor.matmul(be[:], sh_t[u][b][0:1, fsl], ones512[:],
                                         start=False, stop=True)
                        tmp = sb.tile([128, 512], FP32, tag="tmp", name="tmp", bufs=3)
                        nc.vector.tensor_tensor(tmp[:], xc[ft][:], al[:], op=ALU.mult)
                        nc.vector.tensor_tensor(
                            dst_tiles[n][ft // 2][:, ft % 2:ft % 2 + 1, :], tmp[:],
                            be[:], op=ALU.add)

        # ---------------- phase 1: LN1 -> xn8; qkv ----------------
        xn_p = tc.alloc_tile_pool(name="xnp", bufs=1)
        xn8 = [[xn_p.tile([128, 2, 512], FP8, tag=f"xn{n}_{j}", name=f"xn{n}_{j}")
                for j in range(NJH)] for n in range(NTC)]
        _DBG_REFS["xn"] = xn8
        for n in range(NTC):
            nc.vector.memset(xn8[n][NJH - 1][:, 1:2, :], 0.0)
        ln_modulate(xT.ap(), 0, xn8)

        if PHASES >= 3:
            with ExitStack() as ph:
                sb = ph.enter_context(tc.tile_pool(name="qksb", bufs=3))
                wp = ph.enter_context(tc.tile_pool(name="qkw", bufs=1))
                ip = ph.enter_context(tc.tile_pool(name="qki", bufs=2))
                ps = ph.enter_context(tc.tile_pool(name="qkps", bufs=2, space="PSUM"))
                st = ph.enter_context(tc.tile_pool(name="qkst", bufs=1, space="PSUM"))
                qstat = [st.tile([64, 512], FP32, tag=f"qs{n}", name=f"qs{n}") for n in range(NTC)]
                wqkt = [wp.tile([128, 2, 2 * H], FP8, tag=f"wqk{j}", name=f"wqk{j}")
                        for j in range(NJH)]
                for j in range(NJH):
                    nc.sync.dma_start(wqkt[j][:], wqk8.ap()[:, j * 4 * H:(j + 1) * 4 * H])
                for mb in range(QKF):
                    i_s = ip.tile([128, 64], BF16, tag="is", name="is")
                    nc.sync.dma_start(i_s[:], inds.ap()[:, mb * 64:(mb + 1) * 64])
                    i_q = ip.tile([128, 64], BF16, tag="iq", name="iq")
                    nc.sync.dma_start(i_q[:], indq.ap()[:, mb * 64:(mb + 1) * 64])
                    for n in range(NTC):
                        nsl = slice(n * 512, (n + 1) * 512)
                        mmo = ps.tile([128, 512], FP32, tag="mmo", name="mmo")
                        for h2 in range(2):
                            osl = slice(h2 * 256, (h2 + 1) * 256)
                            for j in range(NJH):
                                nc.tensor.matmul(mmo[:, osl],
                                                 wqkt[j][:, :, mb * 128:(mb + 1) * 128],
                                                 xn8[n][j][:, :, osl],
                                                 start=(h2 == 0 and j == 0),
                                                 stop=(h2 == 1 and j == NJH - 1),
                                                 perf_mode=DR, skip_group_check=True)
                        qs = sb.tile([128, 512], BF16, tag="qs", name="qs")
                        nc.scalar.activation(qs[:], mmo[:], AF.Identity,
                                             bias=bqk_s[:, mb:mb + 1], scale=ISCL)
                        sq = sb.tile([128, 512], BF16, tag="sq", name="sq")
                        nc.vector.tensor_mul(sq[:], qs[:], qs[:])
                        nc.tensor.matmul(qstat[n][:], i_s[:], qs[:],
                                         start=(mb == 0), stop=False, skip_group_check=True)
                        nc.tensor.matmul(qstat[n][:], i_q[:], sq[:],
                                         start=False, stop=(mb == QKF - 1), skip_group_check=True)
                        nc.gpsimd.dma_start(qk_d[mb * 128:(mb + 1) * 128, nsl], qs[:])
                # stats psum rows: means (qsum 0-15, ksum 16-31), sq-means (32-63)
                for n in range(NTC):
                    nsl = slice(n * 512, (n + 1) * 512)
                    ms64 = sb.tile([64, 512], FP32, tag="ms64", name="ms64")
                    nc.scalar.mul(ms64[:], qstat[n][:], 1.0 / HD)
                    msq = sb.tile([32, 512], FP32, tag="msqh", name="msqh")
                    nc.sync.dma_start(msq[:], ms64[32:64, :])
                    m2 = sb.tile([32, 512], FP32, tag="m2h", name="m2h")
                    nc.vector.tensor_mul(m2[:], ms64[0:32, :], ms64[0:32, :])
                    var = sb.tile([32, 512], FP32, tag="varh", name="varh")
                    nc.vector.tensor_sub(var[:], msq[:], m2[:])
                    sd = sb.tile([32, 512], FP32, tag="sdh", name="sdh")
                    nc.scalar.activation(sd[:], var[:], AF.Sqrt, bias=eps_col[0:32, :], scale=1.0)
                    nc.vector.reciprocal(stq_r[:, nsl], sd[:])
                    nc.vector.tensor_mul(stq_mr[:, nsl], ms64[0:32, :], stq_r[:, nsl])

        # qkv v-part: xn8 stationary -> v token-major (fp8 DoubleRow)
        if PHASES >= 4:
            with ExitStack() as ph:
                sb = ph.enter_context(tc.tile_pool(name="vsb", bufs=3))
                wp = ph.enter_context(tc.tile_pool(name="vw", bufs=1))
                ps = ph.enter_context(tc.tile_pool(name="vps", bufs=2, space="PSUM"))
                wvt = [wp.tile([128, 2, H], FP8, tag=f"wv{j}", name=f"wv{j}")
                       for j in range(NJH)]
                for j in range(NJH):
                    nc.sync.dma_start(wvt[j][:], wv8.ap()[:, j * 2 * H:(j + 1) * 2 * H])
                VCW = [(0, 256), (256, 256), (512, 256), (768, 256), (1024, 128)]
                for tb in range(T // 128):
                    n, tc0 = tb // 4, (tb % 4) * 128
                    tsl = slice(tb * 128, (tb + 1) * 128)
                    for (v0, vw) in VCW:
                        vp = ps.tile([128, 256], FP32, tag="vp", name="vp")
                        for j in range(NJH):
                            nc.tensor.matmul(vp[:, 0:vw],
                                             xn8[n][j][:, :, tc0:tc0 + 128],
                                             wvt[j][:, :, v0:v0 + vw],
                                             start=(j == 0), stop=False,
                                             perf_mode=DR, skip_group_check=True)
                        nc.tensor.matmul(vp[:, 0:vw], ones512_8[0:1, 0:128],
                                         bv8_s[0:1, v0:v0 + vw],
                                         start=False, stop=True, skip_group_check=True)
                        vs = sb.tile([128, 256], FP8, tag="vs", name="vs")
                        nc.vector.tensor_scalar_mul(vs[:, 0:vw], vp[:, 0:vw], ISCL)
                        nc.gpsimd.dma_start(v_d[tsl, v0:v0 + vw], vs[:, 0:vw])

        if DBG != "xn":
            xn_p.release()

        # ---------------- phase 2: attention ----------------
        if PHASES >= 5:
            with ExitStack() as ph:
                ih = ph.enter_context(tc.tile_pool(name="ihp", bufs=1))
                indh_s = ih.tile([32, 32 * HD], FP32R, tag="indh", name="indh")
                nc.sync.dma_start(indh_s[:], indh.ap())
                qp = ph.enter_context(tc.tile_pool(name="aq", bufs=2))
                up = ph.enter_context(tc.tile_pool(name="au", bufs=2))
                vpl = ph.enter_context(tc.tile_pool(name="av", bufs=2))
                ob = ph.enter_context(tc.tile_pool(name="ao", bufs=2))
                pse = ph.enter_context(tc.tile_pool(name="pse", bufs=1, space="PSUM"))
                pss = ph.enter_context(tc.tile_pool(name="pss", bufs=2, space="PSUM"))
                pso = ph.enter_context(tc.tile_pool(name="pso", bufs=2, space="PSUM"))
                psz = ph.enter_context(tc.tile_pool(name="psz", bufs=1, space="PSUM"))
                for b in range(BPC):
                    c0 = b * N
                    for h in range(NH):
                        r0 = h * HD
                        qr = qp.tile([HD, N], BF16, tag="qr", name="qr")
                        nc.sync.dma_start(qr[:], qk_d[r0:r0 + HD, c0:c0 + N])
                        kr = qp.tile([HD, N], BF16, tag="kr", name="kr")
                        nc.sync.dma_start(kr[:], qk_d[H + r0:H + r0 + HD, c0:c0 + N])
                        qn = qp.tile([HD, N], FP32R, tag="qn", name="qn")
                        kn = qp.tile([HD, N], FP32R, tag="kn", name="kn")
                        for q2 in range(2):
                            lsl = slice(q2 * 512, (q2 + 1) * 512)
                            gsl = slice(c0 + q2 * 512, c0 + q2 * 512 + 512)
                            for (dst, src, gi0) in ((qn, qr, 0), (kn, kr, 16)):
                                rp = pse.tile([HD, 512], FP32, tag="rp", name="rp")
                                nc.tensor.matmul(rp[:], indh_s[:, (gi0 + h) * HD:(gi0 + h + 1) * HD],
                                                 stq_r[:, gsl], start=True, stop=True)
                                mp = pse.tile([HD, 512], FP32, tag="mp", name="mp")
                                nc.tensor.matmul(mp[:], indh_s[:, (gi0 + h) * HD:(gi0 + h + 1) * HD],
                                                 stq_mr[:, gsl], start=True, stop=True)
                                tq = qp.tile([HD, 512], FP32, tag="tq", name="tq")
                                nc.vector.tensor_tensor(tq[:], src[:, lsl], rp[:], op=ALU.mult)
                                nc.vector.tensor_tensor(dst[:, lsl], tq[:], mp[:], op=ALU.subtract)
                        vL = []
                        for p in range(4):  # ktok chunk pairs for DoubleRow
                            vt = vpl.tile([128, 2, 128], FP8, tag=f"vL{p}", name=f"vL{p}")
                            for i in range(2):
                                t0 = c0 + (2 * p + i) * 128
                                nc.gpsimd.dma_start(vt[:, i:i + 1, 0:HD],
                                                  v_d[t0:t0 + 128, r0:r0 + HD])
                            nc.vector.memset(vt[:, :, HD:96], 0.0)
                            nc.vector.memset(vt[:, :, 96:97], 1.0)
                            nc.vector.memset(vt[:, :, 97:128], 0.0)
                            vL.append(vt)
                        # u = exp(z - 2) in fp8 (e4m3 max 448; z <= ~6 so safe)
                        ut = up.tile([128, 8, N], FP8, tag="ut", name="ut")
                        for nk in range(8):
                            for q2 in range(2):
                                lsl = slice(q2 * 512, (q2 + 1) * 512)
                                sp = pss.tile([128, 512], FP32, tag="sp", name="sp")
                                nc.tensor.matmul(sp[:], kn[:, nk * 128:(nk + 1) * 128],
                                                 qn[:, lsl], start=True, stop=True)
                                nc.scalar.activation(ut[:, nk:nk + 1, lsl], sp[:], AF.Exp,
                                                     bias=neg2[:], scale=ISQ)
                        for q2 in range(2):
                            op = pso.tile([128, 512], FP32, tag="op", name="op")
                            for c2 in range(2):
                                csl = slice(q2 * 512 + c2 * 256, q2 * 512 + c2 * 256 + 256)
                                for p in range(4):
                                    nc.tensor.matmul(op[:, c2 * 256:c2 * 256 + 256],
                                                     vL[p][:], ut[:, 2 * p:2 * p + 2, csl],
                                                     start=(c2 == 0 and p == 0),
                                                     stop=(c2 == 1 and p == 3),
                                                     perf_mode=DR, skip_group_check=True)
                            osb = ob.tile([HD, 512], FP32R, tag="osb", name="osb")
                            nc.scalar.copy(osb[:], op[0:HD, :])
                            rz97 = ob.tile([128, 512], FP32R, tag="rz97", name="rz97")
                            nc.vector.reciprocal(rz97[96:97, :], op[96:97, :])
                            rz0 = ob.tile([1, 512], FP32R, tag="rz0", name="rz0")
                            nc.sync.dma_start(rz0[:], rz97[96:97, :])
                            rzp = psz.tile([HD, 512], FP32, tag="rzp", name="rzp")
                            nc.tensor.matmul(rzp[:], ones72, rz0[:], start=True, stop=True)
                            o2 = ob.tile([HD, 512], FP8, tag="o2", name="o2")
                            nc.vector.tensor_tensor(o2[:], osb[:], rzp[:], op=ALU.mult)
                            nc.gpsimd.dma_start(o_d[r0:r0 + HD, c0 + q2 * 512:c0 + q2 * 512 + 512],
                                              o2[:])

        stq_p.release()

        # ---------------- phase 3: proj + gated residual -> x2 ----------------
        if PHASES >= 6:
            with ExitStack() as ph:
                opl = ph.enter_context(tc.tile_pool(name="po", bufs=1))
                wp = ph.enter_context(tc.tile_pool(name="pw", bufs=1))
                sb = ph.enter_context(tc.tile_pool(name="psb", bufs=3))
                ps = ph.enter_context(tc.tile_pool(name="pps", bufs=2, space="PSUM"))
                oc8 = [[opl.tile([128, 2, 512], FP8, tag=f"oc{n}_{j}", name=f"oc{n}_{j}")
                        for j in range(NJH)] for n in range(NTC)]
                for n in range(NTC):
                    nc.vector.memset(oc8[n][NJH - 1][:, 1:2, :], 0.0)
                    nsl = slice(n * 512, (n + 1) * 512)
                    for ft in range(FK):
                        nc.gpsimd.dma_start(oc8[n][ft // 2][:, ft % 2:ft % 2 + 1, :],
                                            o_d[ft * 128:(ft + 1) * 128, nsl])
                wprot = [wp.tile([128, 2, H], FP8, tag=f"wp{j}", name=f"wp{j}")
                         for j in range(NJH)]
                for j in range(NJH):
                    nc.sync.dma_start(wprot[j][:], wpro8.ap()[:, j * 2 * H:(j + 1) * 2 * H])
                for mb in range(FK):
                    msl = slice(mb * 128, (mb + 1) * 128)
                    for n in range(NTC):
                        b = n // (NTC // BPC)
                        nsl = slice(n * 512, (n + 1) * 512)
                        mmo = ps.tile([128, 512], FP32, tag="mmo", name="mmo")
                        for h2 in range(2):
                            osl = slice(h2 * 256, (h2 + 1) * 256)
                            for j in range(NJH):
                                nc.tensor.matmul(mmo[:, osl],
                                                 wprot[j][:, :, msl],
                                                 oc8[n][j][:, :, osl],
                                                 start=(h2 == 0 and j == 0), stop=False,
                                                 perf_mode=DR, skip_group_check=True)
                            nc.tensor.matmul(mmo[:, osl], bpro8_s[0:1, msl],
                                             ones512_8[0:1, 0:256],
                                             start=False, stop=(h2 == 1),
                                             skip_group_check=True)
                        xr = sb.tile([128, 512], FP32R, tag="xr", name="xr")
                        nc.gpsimd.dma_start(xr[:], xT.ap()[msl, nsl])
                        x2s = sb.tile([128, 512], BF16, tag="x2s", name="x2s")
                        nc.vector.scalar_tensor_tensor(x2s[:], mmo[:], gcol16[0][b][:, mb:mb + 1],
                                                       xr[:], op0=ALU.mult, op1=ALU.add)
                        nc.sync.dma_start(x2_d[msl, nsl], x2s[:])

        # ---------------- phase 4+5: LN2 -> xn2; fused MLP ----------------
        if PHASES >= 7:
            xn2_p = tc.alloc_tile_pool(name="xn2p", bufs=1)
            xn2_8 = [[xn2_p.tile([128, 2, 512], FP8, tag=f"xn2{n}_{j}", name=f"xn2{n}_{j}")
                      for j in range(NJH)] for n in range(NTC)]
            _DBG_REFS["xn2"] = xn2_8
            for n in range(NTC):
                nc.vector.memset(xn2_8[n][NJH - 1][:, 1:2, :], 0.0)
            ln_modulate(x2_d, 1, xn2_8, src_dt=BF16, ones_col=ocol_b)

            with ExitStack() as ph:
                w1p_ = ph.enter_context(tc.tile_pool(name="m1w", bufs=1))
                w2p_ = ph.enter_context(tc.tile_pool(name="m2w", bufs=1))
                hp = ph.enter_context(tc.tile_pool(name="mhp", bufs=2))
                sb = ph.enter_context(tc.tile_pool(name="msb", bufs=3))
                ps = ph.enter_context(tc.tile_pool(name="mps", bufs=2, space="PSUM"))
                w1t = [w1p_.tile([128, 2, MLP], FP8, tag=f"w1{j}", name=f"w1{j}")
                       for j in range(NJH)]
                for j in range(NJH):
                    nc.sync.dma_start(w1t[j][:], w18.ap()[:, j * 2 * MLP:(j + 1) * 2 * MLP])
                w2t = [w2p_.tile([128, 2, H], FP8, tag=f"w2{j}", name=f"w2{j}")
                       for j in range(NJM)]
                for j in range(NJM):
                    nc.sync.dma_start(w2t[j][:], w28.ap()[:, j * 2 * H:(j + 1) * 2 * H])
                for n in range(NTC):
                    b = n // (NTC // BPC)
                    nsl = slice(n * 512, (n + 1) * 512)
                    h8t = [hp.tile([128, 2, 512], FP8, tag=f"h8_{j}", name=f"h8_{j}")
                           for j in range(NJM)]
                    for mb in range(MK):
                        mmo = ps.tile([128, 512], FP32, tag="mmo", name="mmo")
                        for h2 in range(2):
                            osl = slice(h2 * 256, (h2 + 1) * 256)
                            for j in range(NJH):
                                nc.tensor.matmul(mmo[:, osl],
                                                 w1t[j][:, :, mb * 128:(mb + 1) * 128],
                                                 xn2_8[n][j][:, :, osl],
                                                 start=(h2 == 0 and j == 0),
                                                 stop=(h2 == 1 and j == NJH - 1),
                                                 perf_mode=DR, skip_group_check=True)
                        nc.scalar.activation(h8t[mb // 2][:, mb % 2:mb % 2 + 1, :], mmo[:],
                                             AF.Gelu_apprx_tanh,
                                             bias=b1_s[:, mb:mb + 1], scale=ISCL)
                    for mb in range(FK):
                        msl = slice(mb * 128, (mb + 1) * 128)
                        mm2 = ps.tile([128, 512], FP32, tag="mm2", name="mm2")
                        for h2 in range(2):
                            osl = slice(h2 * 256, (h2 + 1) * 256)
                            for j in range(NJM):
                                nc.tensor.matmul(mm2[:, osl],
                                                 w2t[j][:, :, msl],
                                                 h8t[j][:, :, osl],
                                                 start=(h2 == 0 and j == 0), stop=False,
                                                 perf_mode=DR, skip_group_check=True)
                            nc.tensor.matmul(mm2[:, osl], b28_s[0:1, msl],
                                             ones512_8[0:1, 0:256],
                                             start=False, stop=(h2 == 1),
                                             skip_group_check=True)
                        x2r = sb.tile([128, 512], BF16, tag="x2r", name="x2r")
                        nc.gpsimd.dma_start(x2r[:], x2_d[msl, nsl])
                        os_ = sb.tile([128, 512], FP32, tag="os", name="os")
                        nc.vector.scalar_tensor_tensor(os_[:], mm2[:], gcol16[1][b][:, mb:mb + 1],
                                                       x2r[:], op0=ALU.mult, op1=ALU.add)
                        nc.gpsimd.dma_start(out.ap()[msl, nsl], os_[:])
            if DBG != "xn2":
                xn2_p.release()

        # ---------------- debug dumps ----------------
        if DBG is not None:
            with ExitStack() as ph:
                db = ph.enter_context(tc.tile_pool(name="dbg", bufs=2))
                if DBG == "mod":
                    dtile = db.tile([128, 108], FP32, tag="dmp", name="dmp")
                    nc.vector.tensor_copy(dtile[:], _DBG_REFS["modc"][:])
                    nc.gpsimd.dma_start(out.ap()[0:128, 0:108], dtile[:])
                elif DBG in ("xn", "xn2"):
                    src = _DBG_REFS[DBG]
                    for n in range(NTC):
                        nsl = slice(n * 512, (n + 1) * 512)
                        for ft in range(FK):
                            dtile = db.tile([128, 512], FP32, tag="dmp", name="dmp")
                            nc.vector.tensor_copy(dtile[:], src[n][:, ft:ft + 1, :])
                            nc.gpsimd.dma_start(out.ap()[ft * 128:(ft + 1) * 128, nsl],
                                              dtile[:])
                elif DBG == "qk":
                    for ft in range(FK):  # dump q rows only
                        nc.gpsimd.dma_start(out.ap()[ft * 128:(ft + 1) * 128, :],
                                          qk_d[ft * 128:(ft + 1) * 128, :].bitcast(FP32))
                elif DBG == "v":
                    for tb in range(T // 128):  # v_d [T, H] -> out[0:H? dump transposed blocks
                        pass
                elif DBG == "o":
                    for ft in range(FK):
                        for n in range(NTC):
                            nsl = slice(n * 512, (n + 1) * 512)
                            stile = db.tile([128, 512], FP8, tag="dm8", name="dm8")
                            nc.sync.dma_start(stile[:], o_d[ft * 128:(ft + 1) * 128, nsl])
                            dtile = db.tile([128, 512], FP32, tag="dmp", name="dmp")
                            nc.vector.tensor_copy(dtile[:], stile[:])
                            nc.gpsimd.dma_start(out.ap()[ft * 128:(ft + 1) * 128, nsl],
                                              dtile[:])
                elif DBG == "x2":
                    for ft in range(FK):
                        nc.gpsimd.dma_start(out.ap()[ft * 128:(ft + 1) * 128, :],
                                          x2_d[ft * 128:(ft + 1) * 128, :].bitcast(FP32))

    nc.finalize()
    return nc


def _pack_dr(w):
    """[Kin, Cols] fp32 -> fp8 [128, njp*2*Cols]: packed[p, (j, i, c)] =
    w[(2j+i)*128 + p, c], zero-padded along Kin to a multiple of 256."""
    kin, cols = w.shape
    njp = -(-kin // 256)
    wp = np.zeros((njp * 256, cols), np.float32)
    wp[:kin] = w
    t = wp.reshape(njp, 2, 128, cols).transpose(2, 0, 1, 3)  # [p, j, i, c]
    return np.ascontiguousarray(
        t.reshape(128, njp * 2 * cols).astype(ml_dtypes.float8_e4m3))


def _host_inputs(x, c, w_mod, b_mod, w_qkv, b_qkv, g_q, g_k, w_proj, b_proj,
                 w1, b1, w2, b2):
    f32 = np.float32
    bf16 = ml_dtypes.bfloat16
    fp8 = ml_dtypes.float8_e4m3
    w_qkv = np.asarray(w_qkv, f32)

    # bmodc[p, c] = b_mod[c*128+p] (+1 for the (1+sc) chunks: quantities 1, 4)
    bm = np.asarray(b_mod, f32).reshape(54, 128).T.copy()
    for q in (1, 4):
        bm[:, q * 9:(q + 1) * 9] += 1.0

    shared = {
        "wmodb": np.ascontiguousarray(np.asarray(w_mod, f32)).astype(bf16),
        "bmodc": np.ascontiguousarray(bm),
        "wqk8": _pack_dr(w_qkv[:, :2 * H] * SCL),
        "wv8": _pack_dr(w_qkv[:, 2 * H:] * SCL),
        "bqk": np.ascontiguousarray(np.asarray(b_qkv, f32)[:2 * H].reshape(QKF, 128).T),
        "bv8": (np.asarray(b_qkv, f32)[2 * H:] * SCL).reshape(1, H).astype(fp8),
        "wpro8": _pack_dr(np.asarray(w_proj, f32) * SCL),
        "bpro8": (np.asarray(b_proj, f32) * SCL).reshape(1, H).astype(fp8),
        "w18": _pack_dr(np.asarray(w1, f32) * SCL),
        "b1c": np.ascontiguousarray(np.asarray(b1, f32).reshape(MK, 128).T),
        "w28": _pack_dr(np.asarray(w2, f32) * SCL),
        "b28": (np.asarray(b2, f32) * SCL).reshape(1, H).astype(fp8),
    }

    # stat rows: qsum=h, ksum=16+h, qsqsum=32+h, ksqsum=48+h
    ind_s = np.zeros((128, QKF * 64), f32)
    ind_q = np.zeros((128, QKF * 64), f32)
    for mb in range(QKF):
        for f in range(128):
            gf = mb * 128 + f
            if gf < H:
                hh, base = gf // HD, 0
            else:
                hh, base = (gf - H) // HD, 16
            ind_s[f, mb * 64 + base + hh] = 1.0
            ind_q[f, mb * 64 + base + 32 + hh] = 1.0
    shared["inds"] = ind_s.astype(ml_dtypes.bfloat16)
    shared["indq"] = ind_q.astype(ml_dtypes.bfloat16)

    # expand rows (stq_r / stq_mr): rq=h, rk=16+h; one [32,72] block per head-slot
    ih = np.zeros((32, 32 * HD), f32)
    for s in range(32):
        gvec = g_q if s < 16 else g_k
        ih[s, s * HD:(s + 1) * HD] = np.asarray(gvec, f32)
    shared["indh"] = ih
    shared["onesr"] = np.ones((1, 512), f32)
    shared["ones8r"] = np.ones((1, 512), f32).astype(fp8)
    shared["onesc"] = np.ones((128, 1), f32)
    shared["epsc"] = np.full((128, 1), EPS, f32)
    vt = np.zeros((128, 25), f32)
    vt[:, 24] = 1.0
    shared["vtail"] = vt

    in_maps = []
    for core in range(NCORES):
        xs = np.asarray(x[core * BPC:(core + 1) * BPC], f32)   # [2, N, H]
        m = dict(shared)
        m["xT"] = np.ascontiguousarray(xs.reshape(T, H).T)
        m["cT"] = np.ascontiguousarray(np.asarray(c[core * BPC:(core + 1) * BPC], f32).T)
        in_maps.append(m)
    return in_maps


def kernel(**inputs):
    if "nc" not in _CACHE:
        _CACHE["nc"] = _build_program()
    nc = _CACHE["nc"]
    in_maps = _host_inputs(**inputs)
    res = run_bass_kernel_spmd(nc, in_maps, core_ids=list(range(NCORES)))
    outs = [res.results[core]["out"].T.reshape(BPC, N, H) for core in range(NCORES)]
    return np.concatenate(outs, axis=0).astype(np.float32)
